# revision 9
# baseline (speedup 1.0000x reference)
"""EnhancedGAE (3-layer GCN encoder + inner-product decoder) on 8 trn2 NeuronCores.

Strategy:
  - Nodes are row-partitioned: core c owns nodes [2048c, 2048(c+1)).
  - Edges (incl. self-loops) are sorted by destination and bucketed per
    128-node destination tile; every tile's list is padded to a uniform
    NBLK*128 so all cores run the same program.
  - Per conv: the (dinv-prescaled) feature table lives in HBM; source rows
    are fetched with dma_gather; the segment-sum over each 128-edge block
    is one TensorE matmul against an indicator matrix S built on the fly
    from iota + is_equal (norm factors are folded into the tables as
    dinv[src] and into the output as dinv[dst]).
  - Layer tables are exchanged with an AllGather collective.
  - adj_recon = sigmoid(z @ z.T) is computed from a transposed z table
    (allgathered), written out as [2048, 16384] row blocks per core.
"""

import sys

sys.path.insert(0, "/opt/trn_rl_repo")

import numpy as np
from concourse import bass, mybir, tile, bacc
from concourse import bass_utils

FP = mybir.dt.float32
I16 = mybir.dt.int16
I32 = mybir.dt.int32

N = 16384
E = 524288
IN = 128
H1 = 256
H2 = 128
LAT = 48
LATP = 64
NCORES = 8
OWN = N // NCORES          # 2048 nodes per core
TPC = OWN // 128           # 16 tiles per core
NT = N // 128              # 128 tiles globally
EPS = 1e-5

AF = mybir.ActivationFunctionType
ALU = mybir.AluOpType

_COMPILED = {}


def _build(nblk: int, dpb2_val: float, phases: int = 4):
    """Build the single-launch SPMD Bass program. nblk = edge blocks per tile."""
    nc = bacc.Bacc(
        "TRN2",
        target_bir_lowering=False,
        debug=False,
        enable_asserts=False,
        num_devices=NCORES,
    )
    L = nblk * 128  # padded edges per destination tile

    # ---- inputs ----
    x_in = nc.dram_tensor("x", [N, IN], FP, kind="ExternalInput")
    xT_own = nc.dram_tensor("xT_own", [128, OWN], FP, kind="ExternalInput")
    deg_pt_in = nc.dram_tensor("deg_pt", [128, NT], FP, kind="ExternalInput")
    deg_own_in = nc.dram_tensor("deg_own", [128, TPC], FP, kind="ExternalInput")
    w1_in = nc.dram_tensor("W1", [IN, H1], FP, kind="ExternalInput")
    w2_in = nc.dram_tensor("W2", [H1, H2], FP, kind="ExternalInput")
    w3_in = nc.dram_tensor("W3p", [H2, LATP], FP, kind="ExternalInput")
    skw_in = nc.dram_tensor("skWp", [IN, LATP], FP, kind="ExternalInput")
    sc1_in = nc.dram_tensor("sc1", [128, H1], FP, kind="ExternalInput")
    sh1_in = nc.dram_tensor("sh1", [128, H1], FP, kind="ExternalInput")
    b2b_in = nc.dram_tensor("b2b", [128, H2], FP, kind="ExternalInput")
    zbb_in = nc.dram_tensor("zbb", [128, LATP], FP, kind="ExternalInput")
    dw1_in = nc.dram_tensor("dW1p", [LATP, LATP], FP, kind="ExternalInput")
    db1_in = nc.dram_tensor("db1b", [128, LATP], FP, kind="ExternalInput")
    dw2_in = nc.dram_tensor("dw2b", [128, LATP], FP, kind="ExternalInput")
    src_in = nc.dram_tensor("src16", [128, TPC * nblk * 8], I16, kind="ExternalInput")
    dst_in = nc.dram_tensor("dstf", [128, TPC * nblk], FP, kind="ExternalInput")

    # ---- outputs ----
    z_out = nc.dram_tensor("z_out", [OWN, LAT], FP, kind="ExternalOutput")
    adj_out = nc.dram_tensor("adj_out", [OWN, N], FP, kind="ExternalOutput")
    dp_out = nc.dram_tensor("dp_out", [OWN, 1], FP, kind="ExternalOutput")

    groups = [list(range(NCORES))]

    with tile.TileContext(nc) as tc:
        with (
            tc.tile_pool(name="const", bufs=1) as const,
            tc.tile_pool(name="dram", bufs=1, space="DRAM") as dram,
        ):
            # persistent DRAM scratch
            xt_tab = dram.tile([N, IN], FP, tag="xt_tab")
            m2_bounce = dram.tile([OWN, H2], FP, tag="m2b")
            m2_tab = dram.tile([N, H2], FP, tag="m2t")
            m3_bounce = dram.tile([OWN, LATP], FP, tag="m3b")
            m3_tab = dram.tile([N, LATP], FP, tag="m3t")
            zT_bounce = dram.tile([LATP, OWN], FP, tag="ztb")
            zT_cat = dram.tile([NCORES * LATP, OWN], FP, tag="ztc")

            # ---- constants in SBUF ----
            iota_f = const.tile([128, 128], FP, tag="iota")
            nc.gpsimd.iota(
                iota_f[:], pattern=[[1, 128]], base=0, channel_multiplier=0,
                allow_small_or_imprecise_dtypes=True,
            )
            iota_pm = const.tile([128, 128], I32, tag="iotapm")
            nc.gpsimd.iota(
                iota_pm[:], pattern=[[-1, 128]], base=0, channel_multiplier=1
            )
            ident = const.tile([128, 128], FP, tag="ident")
            nc.vector.tensor_scalar(ident[:], iota_pm[:], 0, None, op0=ALU.is_equal)

            w1_sb = const.tile([128, H1], FP, tag="w1")
            nc.sync.dma_start(w1_sb[:], w1_in[:])
            w2_sb = const.tile([128, 2, H2], FP, tag="w2")
            nc.sync.dma_start(w2_sb[:], w2_in.rearrange("(k p) n -> p k n", p=128))
            w3_sb = const.tile([128, LATP], FP, tag="w3")
            nc.sync.dma_start(w3_sb[:], w3_in[:])
            skw_sb = const.tile([128, LATP], FP, tag="skw")
            nc.sync.dma_start(skw_sb[:], skw_in[:])
            sc1_sb = const.tile([128, H1], FP, tag="sc1")
            nc.sync.dma_start(sc1_sb[:], sc1_in[:])
            sh1_sb = const.tile([128, H1], FP, tag="sh1")
            nc.sync.dma_start(sh1_sb[:], sh1_in[:])
            b2b_sb = const.tile([128, H2], FP, tag="b2b")
            nc.sync.dma_start(b2b_sb[:], b2b_in[:])
            zbb_sb = const.tile([128, LATP], FP, tag="zbb")
            nc.sync.dma_start(zbb_sb[:], zbb_in[:])
            dw1_sb = const.tile([LATP, LATP], FP, tag="dw1")
            nc.sync.dma_start(dw1_sb[:], dw1_in[:])
            db1_sb = const.tile([128, LATP], FP, tag="db1")
            nc.sync.dma_start(db1_sb[:], db1_in[:])
            dw2_sb = const.tile([128, LATP], FP, tag="dw2")
            nc.sync.dma_start(dw2_sb[:], dw2_in[:])
            xTo_sb = const.tile([128, OWN], FP, tag="xto")
            nc.sync.dma_start(xTo_sb[:], xT_own[:])
            src_sb = const.tile([128, TPC * nblk * 8], I16, tag="src")
            nc.sync.dma_start(src_sb[:], src_in[:])
            dst_sb = const.tile([128, TPC * nblk], FP, tag="dst")
            nc.sync.dma_start(dst_sb[:], dst_in[:])

            deg_sb = const.tile([128, NT], FP, tag="deg")
            nc.sync.dma_start(deg_sb[:], deg_pt_in[:])
            dinv_pt = const.tile([128, NT], FP, tag="dinvpt")
            nc.vector.reciprocal(dinv_pt[:], deg_sb[:])
            nc.scalar.activation(dinv_pt[:], dinv_pt[:], AF.Sqrt)

            dego_sb = const.tile([128, TPC], FP, tag="dego")
            nc.sync.dma_start(dego_sb[:], deg_own_in[:])
            dinv_own = const.tile([128, TPC], FP, tag="dinvown")
            nc.vector.reciprocal(dinv_own[:], dego_sb[:])
            nc.scalar.activation(dinv_own[:], dinv_own[:], AF.Sqrt)

            zT_own = const.tile([LATP, TPC, 128], FP, tag="ztown")

            # ---- build x~ table: xt_tab[v] = dinv[v] * x[v] ----
            with tc.tile_pool(name="xprep", bufs=4) as xp:
                for t in range(NT):
                    xt = xp.tile([128, IN], FP, tag="xt")
                    nc.sync.dma_start(xt[:], x_in[t * 128:(t + 1) * 128, :])
                    xs = xp.tile([128, IN], FP, tag="xs")
                    nc.vector.tensor_scalar(
                        xs[:], xt[:], dinv_pt[:, t:t + 1], None, op0=ALU.mult
                    )
                    nc.sync.dma_start(xt_tab[t * 128:(t + 1) * 128, :], xs[:])

            # ---- generic aggregation: psum_agg += S_b^T @ gathered_b ----
            def aggregate(sb, ps, tab, width, t):
                g = sb.tile([128, nblk, width], FP, tag="gather")
                nc.gpsimd.dma_gather(
                    g[:],
                    tab[:],
                    src_sb[:, t * nblk * 8:(t + 1) * nblk * 8],
                    L,
                    L,
                    width,
                    single_packet=False,
                )
                agg = ps.tile([128, width], FP, tag="agg")
                for b in range(nblk):
                    S = sb.tile([128, 128], FP, tag="S")
                    nc.vector.tensor_scalar(
                        S[:],
                        iota_f[:],
                        dst_sb[:, t * nblk + b:t * nblk + b + 1],
                        None,
                        op0=ALU.is_equal,
                    )
                    nc.tensor.matmul(
                        agg[:], S[:], g[:, b, :],
                        start=(b == 0), stop=(b == nblk - 1),
                    )
                return agg

            # ---- conv1: agg x~ -> @W1 -> BN+ReLU -> @W2 -> m2~ ----
            with (
                tc.tile_pool(name="c1sb", bufs=3) as sb,
                tc.tile_pool(name="c1ps", bufs=2, space="PSUM") as ps,
            ):
                for t in range(TPC):
                    agg = aggregate(sb, ps, xt_tab, IN, t)
                    aggs = sb.tile([128, IN], FP, tag="aggs")
                    nc.vector.tensor_scalar(
                        aggs[:], agg[:], dinv_own[:, t:t + 1], None, op0=ALU.mult
                    )
                    aT_ps = ps.tile([128, 128], FP, tag="tp")
                    nc.tensor.transpose(aT_ps[:], aggs[:], ident[:])
                    aT = sb.tile([128, 128], FP, tag="aT")
                    nc.scalar.activation(aT[:], aT_ps[:], AF.Copy)
                    h1ps = ps.tile([128, H1], FP, tag="h1ps")
                    nc.tensor.matmul(h1ps[:], aT[:], w1_sb[:], start=True, stop=True)
                    t1 = sb.tile([128, H1], FP, tag="t1")
                    nc.vector.tensor_mul(t1[:], h1ps[:], sc1_sb[:])
                    t2 = sb.tile([128, H1], FP, tag="t2")
                    nc.vector.tensor_add(t2[:], t1[:], sh1_sb[:])
                    h1 = sb.tile([128, H1], FP, tag="h1")
                    nc.scalar.activation(h1[:], t2[:], AF.Relu)
                    m2ps = ps.tile([128, H2], FP, tag="m2ps")
                    for k in range(2):
                        hT_ps = ps.tile([128, 128], FP, tag="tp")
                        nc.tensor.transpose(
                            hT_ps[:], h1[:, k * 128:(k + 1) * 128], ident[:]
                        )
                        hT = sb.tile([128, 128], FP, tag="hT")
                        nc.scalar.activation(hT[:], hT_ps[:], AF.Copy)
                        nc.tensor.matmul(
                            m2ps[:], hT[:], w2_sb[:, k, :],
                            start=(k == 0), stop=(k == 1),
                        )
                    m2s = sb.tile([128, H2], FP, tag="m2s")
                    nc.vector.tensor_scalar(
                        m2s[:], m2ps[:], dinv_own[:, t:t + 1], None, op0=ALU.mult
                    )
                    nc.sync.dma_start(m2_bounce[t * 128:(t + 1) * 128, :], m2s[:])

            if phases >= 2:
                nc.gpsimd.collective_compute(
                    "AllGather", ALU.bypass, replica_groups=groups,
                    ins=[m2_bounce.opt()], outs=[m2_tab.opt()],
                )

                # ---- conv2: agg m2~ -> +b2,ReLU -> @W3 -> m3~ ----
                with (
                    tc.tile_pool(name="c2sb", bufs=3) as sb,
                    tc.tile_pool(name="c2ps", bufs=2, space="PSUM") as ps,
                ):
                    for t in range(TPC):
                        agg = aggregate(sb, ps, m2_tab, H2, t)
                        a1 = sb.tile([128, H2], FP, tag="a1")
                        nc.vector.tensor_scalar(
                            a1[:], agg[:], dinv_own[:, t:t + 1], None, op0=ALU.mult
                        )
                        a2 = sb.tile([128, H2], FP, tag="a2")
                        nc.vector.tensor_add(a2[:], a1[:], b2b_sb[:])
                        h2 = sb.tile([128, H2], FP, tag="h2")
                        nc.scalar.activation(h2[:], a2[:], AF.Relu)
                        hT_ps = ps.tile([128, 128], FP, tag="tp")
                        nc.tensor.transpose(hT_ps[:], h2[:], ident[:])
                        hT = sb.tile([128, 128], FP, tag="hT")
                        nc.scalar.activation(hT[:], hT_ps[:], AF.Copy)
                        m3ps = ps.tile([128, LATP], FP, tag="m3ps")
                        nc.tensor.matmul(
                            m3ps[:], hT[:], w3_sb[:], start=True, stop=True
                        )
                        m3s = sb.tile([128, LATP], FP, tag="m3s")
                        nc.vector.tensor_scalar(
                            m3s[:], m3ps[:], dinv_own[:, t:t + 1], None, op0=ALU.mult
                        )
                        nc.sync.dma_start(
                            m3_bounce[t * 128:(t + 1) * 128, :], m3s[:]
                        )

            if phases >= 3:
                nc.gpsimd.collective_compute(
                    "AllGather", ALU.bypass, replica_groups=groups,
                    ins=[m3_bounce.opt()], outs=[m3_tab.opt()],
                )

                # ---- conv3: agg m3~ -> +zb + identity -> z; degree head ----
                with (
                    tc.tile_pool(name="c3sb", bufs=3) as sb,
                    tc.tile_pool(name="c3ps", bufs=2, space="PSUM") as ps,
                ):
                    for t in range(TPC):
                        agg = aggregate(sb, ps, m3_tab, LATP, t)
                        za = sb.tile([128, LATP], FP, tag="za")
                        nc.vector.tensor_scalar(
                            za[:], agg[:], dinv_own[:, t:t + 1], None, op0=ALU.mult
                        )
                        idps = ps.tile([128, LATP], FP, tag="idps")
                        nc.tensor.matmul(
                            idps[:], xTo_sb[:, t * 128:(t + 1) * 128], skw_sb[:],
                            start=True, stop=True,
                        )
                        zb = sb.tile([128, LATP], FP, tag="zb")
                        nc.vector.tensor_add(zb[:], za[:], idps[:])
                        zt = sb.tile([128, LATP], FP, tag="zt")
                        nc.vector.tensor_add(zt[:], zb[:], zbb_sb[:])
                        nc.sync.dma_start(
                            z_out[t * 128:(t + 1) * 128, :], zt[:, :LAT]
                        )
                        zT_ps = ps.tile([LATP, 128], FP, tag="ztps")
                        nc.tensor.transpose(zT_ps[:], zt[:], ident[:])
                        nc.scalar.activation(zT_own[:, t, :], zT_ps[:], AF.Copy)
                        nc.sync.dma_start(
                            zT_bounce[:, t * 128:(t + 1) * 128], zT_own[:, t, :]
                        )
                        # degree head: relu(z @ dpW1 + dpb1) @ dpW2 + dpb2
                        hd_ps = ps.tile([128, LATP], FP, tag="hdps")
                        nc.tensor.matmul(
                            hd_ps[:], zT_own[:, t, :], dw1_sb[:],
                            start=True, stop=True,
                        )
                        hd1 = sb.tile([128, LATP], FP, tag="hd1")
                        nc.vector.tensor_add(hd1[:], hd_ps[:], db1_sb[:])
                        hd = sb.tile([128, LATP], FP, tag="hd")
                        nc.scalar.activation(hd[:], hd1[:], AF.Relu)
                        hm = sb.tile([128, LATP], FP, tag="hm")
                        nc.vector.tensor_mul(hm[:], hd[:], dw2_sb[:])
                        dp = sb.tile([128, 1], FP, tag="dp")
                        nc.vector.tensor_reduce(
                            dp[:], hm[:], axis=mybir.AxisListType.X, op=ALU.add
                        )
                        dp2 = sb.tile([128, 1], FP, tag="dp2")
                        nc.vector.tensor_scalar(
                            dp2[:], dp[:], float(dpb2_val), None, op0=ALU.add
                        )
                        nc.sync.dma_start(
                            dp_out[t * 128:(t + 1) * 128, :], dp2[:]
                        )

            if phases >= 4:
                nc.gpsimd.collective_compute(
                    "AllGather", ALU.bypass, replica_groups=groups,
                    ins=[zT_bounce.opt()], outs=[zT_cat.opt()],
                )

                # ---- adj_recon = sigmoid(z @ z.T), row block per core ----
                with (
                    tc.tile_pool(name="a4sb", bufs=3) as sb,
                    tc.tile_pool(name="a4ps", bufs=4, space="PSUM") as ps,
                ):
                    zT_all = const.tile([LATP, NCORES, OWN], FP, tag="ztall")
                    for cb in range(NCORES):
                        nc.sync.dma_start(
                            zT_all[:, cb, :], zT_cat[cb * LATP:(cb + 1) * LATP, :]
                        )
                    import os as _os
                    _adjn = int(_os.environ.get("GAE_ADJ_TILES", str(TPC)))
                    for r in range(_adjn):
                        for cb in range(NCORES):
                            orow = sb.tile([128, OWN], FP, tag="orow")
                            for j in range(OWN // 512):
                                aps = ps.tile([128, 512], FP, tag="adjps")
                                nc.tensor.matmul(
                                    aps[:],
                                    zT_own[:, r, :],
                                    zT_all[:, cb, j * 512:(j + 1) * 512],
                                    start=True, stop=True,
                                )
                                nc.scalar.activation(
                                    orow[:, j * 512:(j + 1) * 512], aps[:],
                                    AF.Sigmoid,
                                )
                            nc.sync.dma_start(
                                adj_out[
                                    r * 128:(r + 1) * 128, cb * OWN:(cb + 1) * OWN
                                ],
                                orow[:],
                            )

    nc.compile()
    return nc


def _prep_host(x, edge_index, W1, b1, W2, b2, W3, b3, skip_W, skip_b,
               bn_gamma, bn_beta, bn_mean, bn_var, dpW1, dpb1, dpW2, dpb2):
    x = np.asarray(x, np.float32)
    ei = np.asarray(edge_index)
    loops = np.arange(N, dtype=ei.dtype)
    src = np.concatenate([ei[0], loops]).astype(np.int64)
    dst = np.concatenate([ei[1], loops]).astype(np.int64)
    deg = np.bincount(dst, minlength=N).astype(np.float32)

    order = np.argsort(dst, kind="stable")
    ssrc = src[order]
    sdst = dst[order]
    tile_of = sdst >> 7
    counts = np.bincount(tile_of, minlength=NT)
    nblk = int(np.ceil(counts.max() / 128))
    L = nblk * 128

    # per destination tile: padded src (int16) and local dst (f32, 999 pad)
    src_pad = np.zeros((NT, L), np.int16)
    dst_pad = np.full((NT, L), 999.0, np.float32)
    starts = np.zeros(NT + 1, np.int64)
    np.cumsum(counts, out=starts[1:])
    for t in range(NT):
        s, e = starts[t], starts[t + 1]
        cnt = e - s
        src_pad[t, :cnt] = ssrc[s:e].astype(np.int16)
        dst_pad[t, :cnt] = (sdst[s:e] - t * 128).astype(np.float32)

    W1 = np.asarray(W1, np.float32)
    W2 = np.asarray(W2, np.float32)
    W3 = np.asarray(W3, np.float32)
    skip_W = np.asarray(skip_W, np.float32)
    sc1 = (np.asarray(bn_gamma) / np.sqrt(np.asarray(bn_var) + EPS)).astype(np.float32)
    sh1 = ((np.asarray(b1) - np.asarray(bn_mean)) * sc1 + np.asarray(bn_beta)).astype(np.float32)
    W3p = np.zeros((H2, LATP), np.float32)
    W3p[:, :LAT] = W3
    skWp = np.zeros((IN, LATP), np.float32)
    skWp[:, :LAT] = skip_W
    zb = np.zeros(LATP, np.float32)
    zb[:LAT] = np.asarray(b3, np.float32) + np.asarray(skip_b, np.float32)
    dW1p = np.zeros((LATP, LATP), np.float32)
    dW1p[:LAT, :] = np.asarray(dpW1, np.float32)
    db1 = np.asarray(dpb1, np.float32)
    dw2 = np.asarray(dpW2, np.float32)[:, 0]
    dpb2_val = float(np.asarray(dpb2)[0])

    def bcast(v, w):
        return np.broadcast_to(np.asarray(v, np.float32)[None, :], (128, w)).copy()

    deg_pt = deg.reshape(NT, 128).T.copy()

    common = {
        "x": x,
        "deg_pt": deg_pt,
        "W1": W1,
        "W2": W2,
        "W3p": W3p,
        "skWp": skWp,
        "sc1": bcast(sc1, H1),
        "sh1": bcast(sh1, H1),
        "b2b": bcast(b2, H2),
        "zbb": bcast(zb, LATP),
        "dW1p": dW1p,
        "db1b": bcast(db1, LATP),
        "dw2b": bcast(dw2, LATP),
    }

    in_maps = []
    for c in range(NCORES):
        t0 = c * TPC
        sp = src_pad[t0:t0 + TPC].reshape(-1)          # [TPC*L]
        dp = dst_pad[t0:t0 + TPC]                      # [TPC, L]
        m = dict(common)
        m["xT_own"] = x[c * OWN:(c + 1) * OWN].T.copy()
        m["deg_own"] = deg_pt[:, t0:t0 + TPC].copy()
        # wrapped in 16 partitions, replicated for each of the 8 Q7 cores
        m["src16"] = np.tile(sp.reshape(-1, 16).T, (8, 1)).copy()  # [128, TPC*L/16]
        m["dstf"] = dp.reshape(TPC * nblk, 128).T.copy()  # [128, TPC*nblk]
        in_maps.append(m)
    return nblk, dpb2_val, in_maps


def _run(nc, in_maps):
    return bass_utils.run_bass_kernel_spmd(
        nc, in_maps, core_ids=list(range(NCORES))
    )


def kernel(**inputs):
    import os

    phases = int(os.environ.get("GAE_PHASES", "4"))
    nblk, dpb2_val, in_maps = _prep_host(**inputs)
    key = (nblk, dpb2_val, phases)
    if key not in _COMPILED:
        _COMPILED[key] = _build(nblk, dpb2_val, phases)
    nc = _COMPILED[key]
    res = _run(nc, in_maps)
    z = np.concatenate([res.results[c]["z_out"] for c in range(NCORES)], axis=0)
    adj = np.concatenate([res.results[c]["adj_out"] for c in range(NCORES)], axis=0)
    dp = np.concatenate(
        [res.results[c]["dp_out"][:, 0] for c in range(NCORES)], axis=0
    )
    return z, adj, dp


# revision 14
# speedup vs baseline: 1.3318x; 1.3318x over previous
"""EnhancedGAE (3-layer GCN encoder + inner-product decoder) on 8 trn2 NeuronCores.

Strategy:
  - Nodes are row-partitioned: core c owns nodes [2048c, 2048(c+1)).
  - Edges (incl. self-loops) are sorted by destination and bucketed per
    128-node destination tile; every tile's list is padded to a uniform
    NBLK*128 so all cores run the same program.
  - Per conv: the (dinv-prescaled) feature table lives in HBM; source rows
    are fetched with dma_gather; the segment-sum over each 128-edge block
    is one TensorE matmul against an indicator matrix S built on the fly
    from iota + is_equal (norm factors are folded into the tables as
    dinv[src] and into the output as dinv[dst]).
  - Layer tables are exchanged with an AllGather collective.
  - adj_recon = sigmoid(z @ z.T) is computed from a transposed z table
    (allgathered), written out as [2048, 16384] row blocks per core.
"""

import sys

sys.path.insert(0, "/opt/trn_rl_repo")

import numpy as np
from concourse import bass, mybir, tile, bacc
from concourse import bass_utils

FP = mybir.dt.float32
I16 = mybir.dt.int16
I32 = mybir.dt.int32

N = 16384
E = 524288
IN = 128
H1 = 256
H2 = 128
LAT = 48
LATP = 64
NCORES = 8
OWN = N // NCORES          # 2048 nodes per core
TPC = OWN // 128           # 16 tiles per core
NT = N // 128              # 128 tiles globally
EPS = 1e-5

AF = mybir.ActivationFunctionType
ALU = mybir.AluOpType

_COMPILED = {}


def _build(nblk: int, dpb2_val: float, phases: int = 4):
    """Build the single-launch SPMD Bass program. nblk = edge blocks per tile."""
    nc = bacc.Bacc(
        "TRN2",
        target_bir_lowering=False,
        debug=False,
        enable_asserts=False,
        num_devices=NCORES,
    )
    L = nblk * 128  # padded edges per destination tile

    # ---- inputs ----
    x_in = nc.dram_tensor("x", [N, IN], FP, kind="ExternalInput")
    xT_own = nc.dram_tensor("xT_own", [128, OWN], FP, kind="ExternalInput")
    deg_pt_in = nc.dram_tensor("deg_pt", [128, NT], FP, kind="ExternalInput")
    deg_own_in = nc.dram_tensor("deg_own", [128, TPC], FP, kind="ExternalInput")
    w1_in = nc.dram_tensor("W1", [IN, H1], FP, kind="ExternalInput")
    w2_in = nc.dram_tensor("W2", [H1, H2], FP, kind="ExternalInput")
    w3_in = nc.dram_tensor("W3p", [H2, LATP], FP, kind="ExternalInput")
    skw_in = nc.dram_tensor("skWp", [IN, LATP], FP, kind="ExternalInput")
    sc1_in = nc.dram_tensor("sc1", [128, H1], FP, kind="ExternalInput")
    sh1_in = nc.dram_tensor("sh1", [128, H1], FP, kind="ExternalInput")
    b2b_in = nc.dram_tensor("b2b", [128, H2], FP, kind="ExternalInput")
    zbb_in = nc.dram_tensor("zbb", [128, LATP], FP, kind="ExternalInput")
    dw1_in = nc.dram_tensor("dW1p", [LATP, LATP], FP, kind="ExternalInput")
    db1_in = nc.dram_tensor("db1b", [128, LATP], FP, kind="ExternalInput")
    dw2_in = nc.dram_tensor("dw2b", [128, LATP], FP, kind="ExternalInput")
    src_in = nc.dram_tensor("src16", [128, TPC * nblk * 8], I16, kind="ExternalInput")
    dst_in = nc.dram_tensor("dstf", [128, TPC * nblk], FP, kind="ExternalInput")

    # ---- outputs ----
    z_out = nc.dram_tensor("z_out", [OWN, LAT], FP, kind="ExternalOutput")
    adj_out = nc.dram_tensor("adj_out", [OWN, N], FP, kind="ExternalOutput")
    dp_out = nc.dram_tensor("dp_out", [OWN, 1], FP, kind="ExternalOutput")

    groups = [list(range(NCORES))]

    with tile.TileContext(nc) as tc:
        with (
            tc.tile_pool(name="const", bufs=1) as const,
            tc.tile_pool(name="dram", bufs=1, space="DRAM") as dram,
        ):
            # persistent DRAM scratch
            xt_tab = dram.tile([N, IN], FP, tag="xt_tab")
            m2_bounce = dram.tile([OWN, H2], FP, tag="m2b")
            m2_tab = dram.tile([N, H2], FP, tag="m2t")
            m3_bounce = dram.tile([OWN, LATP], FP, tag="m3b")
            m3_tab = dram.tile([N, LATP], FP, tag="m3t")
            BF = mybir.dt.bfloat16
            zT_bounce_hi = dram.tile([LATP, OWN], BF, tag="ztbh")
            zT_bounce_lo = dram.tile([LATP, OWN], BF, tag="ztbl")
            zT_cat_hi = dram.tile([NCORES * LATP, OWN], BF, tag="ztch")
            zT_cat_lo = dram.tile([NCORES * LATP, OWN], BF, tag="ztcl")

            # ---- constants in SBUF ----
            iota_f = const.tile([128, 128], FP, tag="iota")
            nc.gpsimd.iota(
                iota_f[:], pattern=[[1, 128]], base=0, channel_multiplier=0,
                allow_small_or_imprecise_dtypes=True,
            )
            iota_pm = const.tile([128, 128], I32, tag="iotapm")
            nc.gpsimd.iota(
                iota_pm[:], pattern=[[-1, 128]], base=0, channel_multiplier=1
            )
            ident = const.tile([128, 128], FP, tag="ident")
            nc.vector.tensor_scalar(ident[:], iota_pm[:], 0, None, op0=ALU.is_equal)

            w1_sb = const.tile([128, H1], FP, tag="w1")
            nc.sync.dma_start(w1_sb[:], w1_in[:])
            w2_sb = const.tile([128, 2, H2], FP, tag="w2")
            nc.sync.dma_start(w2_sb[:], w2_in.rearrange("(k p) n -> p k n", p=128))
            w3_sb = const.tile([128, LATP], FP, tag="w3")
            nc.sync.dma_start(w3_sb[:], w3_in[:])
            skw_sb = const.tile([128, LATP], FP, tag="skw")
            nc.sync.dma_start(skw_sb[:], skw_in[:])
            sc1_sb = const.tile([128, H1], FP, tag="sc1")
            nc.sync.dma_start(sc1_sb[:], sc1_in[:])
            sh1_sb = const.tile([128, H1], FP, tag="sh1")
            nc.sync.dma_start(sh1_sb[:], sh1_in[:])
            b2b_sb = const.tile([128, H2], FP, tag="b2b")
            nc.sync.dma_start(b2b_sb[:], b2b_in[:])
            zbb_sb = const.tile([128, LATP], FP, tag="zbb")
            nc.sync.dma_start(zbb_sb[:], zbb_in[:])
            dw1_sb = const.tile([LATP, LATP], FP, tag="dw1")
            nc.sync.dma_start(dw1_sb[:], dw1_in[:])
            db1_sb = const.tile([128, LATP], FP, tag="db1")
            nc.sync.dma_start(db1_sb[:], db1_in[:])
            dw2_sb = const.tile([128, LATP], FP, tag="dw2")
            nc.sync.dma_start(dw2_sb[:], dw2_in[:])
            xTo_sb = const.tile([128, OWN], FP, tag="xto")
            nc.sync.dma_start(xTo_sb[:], xT_own[:])
            src_sb = const.tile([128, TPC * nblk * 8], I16, tag="src")
            nc.sync.dma_start(src_sb[:], src_in[:])
            dst_sb = const.tile([128, TPC * nblk], FP, tag="dst")
            nc.sync.dma_start(dst_sb[:], dst_in[:])

            deg_sb = const.tile([128, NT], FP, tag="deg")
            nc.sync.dma_start(deg_sb[:], deg_pt_in[:])
            dinv_pt = const.tile([128, NT], FP, tag="dinvpt")
            nc.vector.reciprocal(dinv_pt[:], deg_sb[:])
            nc.scalar.activation(dinv_pt[:], dinv_pt[:], AF.Sqrt)

            dego_sb = const.tile([128, TPC], FP, tag="dego")
            nc.sync.dma_start(dego_sb[:], deg_own_in[:])
            dinv_own = const.tile([128, TPC], FP, tag="dinvown")
            nc.vector.reciprocal(dinv_own[:], dego_sb[:])
            nc.scalar.activation(dinv_own[:], dinv_own[:], AF.Sqrt)

            zT_own = const.tile([LATP, TPC, 128], FP, tag="ztown")

            # ---- build x~ table: xt_tab[v] = dinv[v] * x[v] ----
            with tc.tile_pool(name="xprep", bufs=4) as xp:
                for t in range(NT):
                    xt = xp.tile([128, IN], FP, tag="xt")
                    nc.sync.dma_start(xt[:], x_in[t * 128:(t + 1) * 128, :])
                    xs = xp.tile([128, IN], FP, tag="xs")
                    nc.vector.tensor_scalar(
                        xs[:], xt[:], dinv_pt[:, t:t + 1], None, op0=ALU.mult
                    )
                    nc.sync.dma_start(xt_tab[t * 128:(t + 1) * 128, :], xs[:])

            # ---- generic aggregation: psum_agg += S_b^T @ gathered_b ----
            iota_b = iota_f[:].rearrange("p (a f) -> p a f", a=1).broadcast_to(
                (128, nblk, 128)
            )

            def aggregate(sb, ps, tab, width, t):
                g = sb.tile([128, nblk, width], FP, tag="gather")
                nc.gpsimd.dma_gather(
                    g[:],
                    tab[:],
                    src_sb[:, t * nblk * 8:(t + 1) * nblk * 8],
                    L,
                    L,
                    width,
                    single_packet=False,
                )
                # all NBLK selector matrices in one DVE op:
                # S[p, b, j] = (j == dst_local[p, b])
                S = sb.tile([128, nblk, 128], FP, tag="S")
                nc.vector.tensor_tensor(
                    S[:],
                    iota_b,
                    dst_sb[:, t * nblk:(t + 1) * nblk].broadcast_to(
                        (128, nblk, 128)
                    ),
                    op=ALU.is_equal,
                )
                agg = ps.tile([128, width], FP, tag="agg")
                for b in range(nblk):
                    nc.tensor.matmul(
                        agg[:], S[:, b, :], g[:, b, :],
                        start=(b == 0), stop=(b == nblk - 1),
                    )
                return agg

            # ---- conv1: agg x~ -> @W1 -> BN+ReLU -> @W2 -> m2~ ----
            with (
                tc.tile_pool(name="c1sb", bufs=3) as sb,
                tc.tile_pool(name="c1ps", bufs=2, space="PSUM") as ps,
            ):
                for t in range(TPC):
                    agg = aggregate(sb, ps, xt_tab, IN, t)
                    aggs = sb.tile([128, IN], FP, tag="aggs")
                    nc.vector.tensor_scalar(
                        aggs[:], agg[:], dinv_own[:, t:t + 1], None, op0=ALU.mult
                    )
                    aT_ps = ps.tile([128, 128], FP, tag="tp")
                    nc.tensor.transpose(aT_ps[:], aggs[:], ident[:])
                    aT = sb.tile([128, 128], FP, tag="aT")
                    nc.scalar.activation(aT[:], aT_ps[:], AF.Copy)
                    h1ps = ps.tile([128, H1], FP, tag="h1ps")
                    nc.tensor.matmul(h1ps[:], aT[:], w1_sb[:], start=True, stop=True)
                    t1 = sb.tile([128, H1], FP, tag="t1")
                    nc.vector.tensor_mul(t1[:], h1ps[:], sc1_sb[:])
                    t2 = sb.tile([128, H1], FP, tag="t2")
                    nc.vector.tensor_add(t2[:], t1[:], sh1_sb[:])
                    h1 = sb.tile([128, H1], FP, tag="h1")
                    nc.scalar.activation(h1[:], t2[:], AF.Relu)
                    m2ps = ps.tile([128, H2], FP, tag="m2ps")
                    for k in range(2):
                        hT_ps = ps.tile([128, 128], FP, tag="tp")
                        nc.tensor.transpose(
                            hT_ps[:], h1[:, k * 128:(k + 1) * 128], ident[:]
                        )
                        hT = sb.tile([128, 128], FP, tag="hT")
                        nc.scalar.activation(hT[:], hT_ps[:], AF.Copy)
                        nc.tensor.matmul(
                            m2ps[:], hT[:], w2_sb[:, k, :],
                            start=(k == 0), stop=(k == 1),
                        )
                    m2s = sb.tile([128, H2], FP, tag="m2s")
                    nc.vector.tensor_scalar(
                        m2s[:], m2ps[:], dinv_own[:, t:t + 1], None, op0=ALU.mult
                    )
                    nc.sync.dma_start(m2_bounce[t * 128:(t + 1) * 128, :], m2s[:])

            if phases >= 2:
                nc.gpsimd.collective_compute(
                    "AllGather", ALU.bypass, replica_groups=groups,
                    ins=[m2_bounce.opt()], outs=[m2_tab.opt()],
                )

                # ---- conv2: agg m2~ -> +b2,ReLU -> @W3 -> m3~ ----
                with (
                    tc.tile_pool(name="c2sb", bufs=3) as sb,
                    tc.tile_pool(name="c2ps", bufs=2, space="PSUM") as ps,
                ):
                    for t in range(TPC):
                        agg = aggregate(sb, ps, m2_tab, H2, t)
                        a1 = sb.tile([128, H2], FP, tag="a1")
                        nc.vector.tensor_scalar(
                            a1[:], agg[:], dinv_own[:, t:t + 1], None, op0=ALU.mult
                        )
                        a2 = sb.tile([128, H2], FP, tag="a2")
                        nc.vector.tensor_add(a2[:], a1[:], b2b_sb[:])
                        h2 = sb.tile([128, H2], FP, tag="h2")
                        nc.scalar.activation(h2[:], a2[:], AF.Relu)
                        hT_ps = ps.tile([128, 128], FP, tag="tp")
                        nc.tensor.transpose(hT_ps[:], h2[:], ident[:])
                        hT = sb.tile([128, 128], FP, tag="hT")
                        nc.scalar.activation(hT[:], hT_ps[:], AF.Copy)
                        m3ps = ps.tile([128, LATP], FP, tag="m3ps")
                        nc.tensor.matmul(
                            m3ps[:], hT[:], w3_sb[:], start=True, stop=True
                        )
                        m3s = sb.tile([128, LATP], FP, tag="m3s")
                        nc.vector.tensor_scalar(
                            m3s[:], m3ps[:], dinv_own[:, t:t + 1], None, op0=ALU.mult
                        )
                        nc.sync.dma_start(
                            m3_bounce[t * 128:(t + 1) * 128, :], m3s[:]
                        )

            if phases >= 3:
                nc.gpsimd.collective_compute(
                    "AllGather", ALU.bypass, replica_groups=groups,
                    ins=[m3_bounce.opt()], outs=[m3_tab.opt()],
                )

                # ---- conv3: agg m3~ -> +zb + identity -> z; degree head ----
                with (
                    tc.tile_pool(name="c3sb", bufs=3) as sb,
                    tc.tile_pool(name="c3ps", bufs=2, space="PSUM") as ps,
                ):
                    for t in range(TPC):
                        agg = aggregate(sb, ps, m3_tab, LATP, t)
                        za = sb.tile([128, LATP], FP, tag="za")
                        nc.vector.tensor_scalar(
                            za[:], agg[:], dinv_own[:, t:t + 1], None, op0=ALU.mult
                        )
                        idps = ps.tile([128, LATP], FP, tag="idps")
                        nc.tensor.matmul(
                            idps[:], xTo_sb[:, t * 128:(t + 1) * 128], skw_sb[:],
                            start=True, stop=True,
                        )
                        zb = sb.tile([128, LATP], FP, tag="zb")
                        nc.vector.tensor_add(zb[:], za[:], idps[:])
                        zt = sb.tile([128, LATP], FP, tag="zt")
                        nc.vector.tensor_add(zt[:], zb[:], zbb_sb[:])
                        nc.sync.dma_start(
                            z_out[t * 128:(t + 1) * 128, :], zt[:, :LAT]
                        )
                        zT_ps = ps.tile([LATP, 128], FP, tag="ztps")
                        nc.tensor.transpose(zT_ps[:], zt[:], ident[:])
                        nc.scalar.activation(zT_own[:, t, :], zT_ps[:], AF.Copy)
                        # split z^T into bf16 hi + lo for the adj matmuls
                        hi_bf = sb.tile([LATP, 128], BF, tag="hibf")
                        nc.scalar.activation(hi_bf[:], zT_ps[:], AF.Copy)
                        hi32 = sb.tile([LATP, 128], FP, tag="hi32")
                        nc.vector.tensor_copy(hi32[:], hi_bf[:])
                        lo_bf = sb.tile([LATP, 128], BF, tag="lobf")
                        nc.vector.tensor_sub(lo_bf[:], zT_ps[:], hi32[:])
                        nc.sync.dma_start(
                            zT_bounce_hi[:, t * 128:(t + 1) * 128], hi_bf[:]
                        )
                        nc.sync.dma_start(
                            zT_bounce_lo[:, t * 128:(t + 1) * 128], lo_bf[:]
                        )
                        # degree head: relu(z @ dpW1 + dpb1) @ dpW2 + dpb2
                        hd_ps = ps.tile([128, LATP], FP, tag="hdps")
                        nc.tensor.matmul(
                            hd_ps[:], zT_own[:, t, :], dw1_sb[:],
                            start=True, stop=True,
                        )
                        hd1 = sb.tile([128, LATP], FP, tag="hd1")
                        nc.vector.tensor_add(hd1[:], hd_ps[:], db1_sb[:])
                        hd = sb.tile([128, LATP], FP, tag="hd")
                        nc.scalar.activation(hd[:], hd1[:], AF.Relu)
                        hm = sb.tile([128, LATP], FP, tag="hm")
                        nc.vector.tensor_mul(hm[:], hd[:], dw2_sb[:])
                        dp = sb.tile([128, 1], FP, tag="dp")
                        nc.vector.tensor_reduce(
                            dp[:], hm[:], axis=mybir.AxisListType.X, op=ALU.add
                        )
                        dp2 = sb.tile([128, 1], FP, tag="dp2")
                        nc.vector.tensor_scalar(
                            dp2[:], dp[:], float(dpb2_val), None, op0=ALU.add
                        )
                        nc.sync.dma_start(
                            dp_out[t * 128:(t + 1) * 128, :], dp2[:]
                        )

            if phases >= 4:
                nc.gpsimd.collective_compute(
                    "AllGather", ALU.bypass, replica_groups=groups,
                    ins=[zT_bounce_hi.opt()], outs=[zT_cat_hi.opt()],
                )
                nc.gpsimd.collective_compute(
                    "AllGather", ALU.bypass, replica_groups=groups,
                    ins=[zT_bounce_lo.opt()], outs=[zT_cat_lo.opt()],
                )

                # ---- adj_recon = sigmoid(z @ z.T), row block per core ----
                # split-bf16: logits = hi@hi + lo@hi + hi@lo  (lo@lo dropped)
                # MM_A: lhsT=[hi;lo], rhs=[hi;hi];  MM_B: lhsT=[hi;0], rhs=[lo;lo]
                with (
                    tc.tile_pool(name="a4sb", bufs=3) as sb,
                    tc.tile_pool(name="a4c", bufs=1) as a4c,
                    tc.tile_pool(name="a4ps", bufs=8, space="PSUM") as ps,
                ):
                    zall_hh = a4c.tile([128, NCORES, OWN], BF, tag="zallhh")
                    zall_ll = a4c.tile([128, NCORES, OWN], BF, tag="zallll")
                    for cb in range(NCORES):
                        cat_h = zT_cat_hi[cb * LATP:(cb + 1) * LATP, :]
                        cat_l = zT_cat_lo[cb * LATP:(cb + 1) * LATP, :]
                        nc.sync.dma_start(zall_hh[0:LATP, cb, :], cat_h)
                        nc.sync.dma_start(zall_hh[LATP:128, cb, :], cat_h)
                        nc.sync.dma_start(zall_ll[0:LATP, cb, :], cat_l)
                        nc.sync.dma_start(zall_ll[LATP:128, cb, :], cat_l)
                    zown_A = a4c.tile([128, TPC, 128], BF, tag="zownA")
                    zown_B = a4c.tile([128, TPC, 128], BF, tag="zownB")
                    nc.sync.dma_start(
                        zown_A[0:LATP, :, :],
                        zT_bounce_hi[:].rearrange("p (t f) -> p t f", f=128),
                    )
                    nc.sync.dma_start(
                        zown_A[LATP:128, :, :],
                        zT_bounce_lo[:].rearrange("p (t f) -> p t f", f=128),
                    )
                    nc.sync.dma_start(
                        zown_B[0:LATP, :, :],
                        zT_bounce_hi[:].rearrange("p (t f) -> p t f", f=128),
                    )
                    nc.gpsimd.memset(zown_B[LATP:128, :, :], 0.0)

                    NJ = OWN // 512
                    for r in range(TPC):
                        for cb in range(NCORES):
                            orow = sb.tile([128, OWN], FP, tag="orow")
                            apss = []
                            for _j in range(NJ):
                                aps_t = ps.tile([128, 512], FP, tag="adjps")
                                apss.append(aps_t)
                            for j in range(NJ):
                                nc.tensor.matmul(
                                    apss[j][:],
                                    zown_A[:, r, :],
                                    zall_hh[:, cb, j * 512:(j + 1) * 512],
                                    start=True, stop=False,
                                )
                            for j in range(NJ):
                                nc.tensor.matmul(
                                    apss[j][:],
                                    zown_B[:, r, :],
                                    zall_ll[:, cb, j * 512:(j + 1) * 512],
                                    start=False, stop=True,
                                )
                            for j in range(NJ):
                                nc.scalar.activation(
                                    orow[:, j * 512:(j + 1) * 512], apss[j][:],
                                    AF.Sigmoid,
                                )
                            nc.sync.dma_start(
                                adj_out[
                                    r * 128:(r + 1) * 128, cb * OWN:(cb + 1) * OWN
                                ],
                                orow[:],
                            )

    nc.compile()
    return nc


def _prep_host(x, edge_index, W1, b1, W2, b2, W3, b3, skip_W, skip_b,
               bn_gamma, bn_beta, bn_mean, bn_var, dpW1, dpb1, dpW2, dpb2):
    x = np.asarray(x, np.float32)
    ei = np.asarray(edge_index)
    loops = np.arange(N, dtype=ei.dtype)
    src = np.concatenate([ei[0], loops]).astype(np.int64)
    dst = np.concatenate([ei[1], loops]).astype(np.int64)
    deg = np.bincount(dst, minlength=N).astype(np.float32)

    order = np.argsort(dst, kind="stable")
    ssrc = src[order]
    sdst = dst[order]
    tile_of = sdst >> 7
    counts = np.bincount(tile_of, minlength=NT)
    nblk = int(np.ceil(counts.max() / 128))
    L = nblk * 128

    # per destination tile: padded src (int16) and local dst (f32, 999 pad)
    src_pad = np.zeros((NT, L), np.int16)
    dst_pad = np.full((NT, L), 999.0, np.float32)
    starts = np.zeros(NT + 1, np.int64)
    np.cumsum(counts, out=starts[1:])
    for t in range(NT):
        s, e = starts[t], starts[t + 1]
        cnt = e - s
        src_pad[t, :cnt] = ssrc[s:e].astype(np.int16)
        dst_pad[t, :cnt] = (sdst[s:e] - t * 128).astype(np.float32)

    W1 = np.asarray(W1, np.float32)
    W2 = np.asarray(W2, np.float32)
    W3 = np.asarray(W3, np.float32)
    skip_W = np.asarray(skip_W, np.float32)
    sc1 = (np.asarray(bn_gamma) / np.sqrt(np.asarray(bn_var) + EPS)).astype(np.float32)
    sh1 = ((np.asarray(b1) - np.asarray(bn_mean)) * sc1 + np.asarray(bn_beta)).astype(np.float32)
    W3p = np.zeros((H2, LATP), np.float32)
    W3p[:, :LAT] = W3
    skWp = np.zeros((IN, LATP), np.float32)
    skWp[:, :LAT] = skip_W
    zb = np.zeros(LATP, np.float32)
    zb[:LAT] = np.asarray(b3, np.float32) + np.asarray(skip_b, np.float32)
    dW1p = np.zeros((LATP, LATP), np.float32)
    dW1p[:LAT, :] = np.asarray(dpW1, np.float32)
    db1 = np.asarray(dpb1, np.float32)
    dw2 = np.asarray(dpW2, np.float32)[:, 0]
    dpb2_val = float(np.asarray(dpb2)[0])

    def bcast(v, w):
        return np.broadcast_to(np.asarray(v, np.float32)[None, :], (128, w)).copy()

    deg_pt = deg.reshape(NT, 128).T.copy()

    common = {
        "x": x,
        "deg_pt": deg_pt,
        "W1": W1,
        "W2": W2,
        "W3p": W3p,
        "skWp": skWp,
        "sc1": bcast(sc1, H1),
        "sh1": bcast(sh1, H1),
        "b2b": bcast(b2, H2),
        "zbb": bcast(zb, LATP),
        "dW1p": dW1p,
        "db1b": bcast(db1, LATP),
        "dw2b": bcast(dw2, LATP),
    }

    in_maps = []
    for c in range(NCORES):
        t0 = c * TPC
        sp = src_pad[t0:t0 + TPC].reshape(-1)          # [TPC*L]
        dp = dst_pad[t0:t0 + TPC]                      # [TPC, L]
        m = dict(common)
        m["xT_own"] = x[c * OWN:(c + 1) * OWN].T.copy()
        m["deg_own"] = deg_pt[:, t0:t0 + TPC].copy()
        # wrapped in 16 partitions, replicated for each of the 8 Q7 cores
        m["src16"] = np.tile(sp.reshape(-1, 16).T, (8, 1)).copy()  # [128, TPC*L/16]
        m["dstf"] = dp.reshape(TPC * nblk, 128).T.copy()  # [128, TPC*nblk]
        in_maps.append(m)
    return nblk, dpb2_val, in_maps


def _run(nc, in_maps):
    return bass_utils.run_bass_kernel_spmd(
        nc, in_maps, core_ids=list(range(NCORES))
    )


def kernel(**inputs):
    import os

    phases = int(os.environ.get("GAE_PHASES", "4"))
    nblk, dpb2_val, in_maps = _prep_host(**inputs)
    key = (nblk, dpb2_val, phases)
    if key not in _COMPILED:
        _COMPILED[key] = _build(nblk, dpb2_val, phases)
    nc = _COMPILED[key]
    res = _run(nc, in_maps)
    z = np.concatenate([res.results[c]["z_out"] for c in range(NCORES)], axis=0)
    adj = np.concatenate([res.results[c]["adj_out"] for c in range(NCORES)], axis=0)
    dp = np.concatenate(
        [res.results[c]["dp_out"][:, 0] for c in range(NCORES)], axis=0
    )
    return z, adj, dp


# revision 15
# speedup vs baseline: 1.4695x; 1.1034x over previous
"""EnhancedGAE (3-layer GCN encoder + inner-product decoder) on 8 trn2 NeuronCores.

Strategy:
  - Nodes are row-partitioned: core c owns nodes [2048c, 2048(c+1)).
  - Edges (incl. self-loops) are sorted by destination and bucketed per
    128-node destination tile; every tile's list is padded to a uniform
    NBLK*128 so all cores run the same program.
  - Per conv: the (dinv-prescaled) feature table lives in HBM; source rows
    are fetched with dma_gather; the segment-sum over each 128-edge block
    is one TensorE matmul against an indicator matrix S built on the fly
    from iota + is_equal (norm factors are folded into the tables as
    dinv[src] and into the output as dinv[dst]).
  - Layer tables are exchanged with an AllGather collective.
  - adj_recon = sigmoid(z @ z.T) is computed from a transposed z table
    (allgathered), written out as [2048, 16384] row blocks per core.
"""

import sys

sys.path.insert(0, "/opt/trn_rl_repo")

import numpy as np
from concourse import bass, mybir, tile, bacc
from concourse import bass_utils

FP = mybir.dt.float32
I16 = mybir.dt.int16
I32 = mybir.dt.int32

N = 16384
E = 524288
IN = 128
H1 = 256
H2 = 128
LAT = 48
LATP = 64
NCORES = 8
OWN = N // NCORES          # 2048 nodes per core
TPC = OWN // 128           # 16 tiles per core
NT = N // 128              # 128 tiles globally
EPS = 1e-5

AF = mybir.ActivationFunctionType
ALU = mybir.AluOpType

_COMPILED = {}


def _build(nblk: int, dpb2_val: float, phases: int = 4):
    """Build the single-launch SPMD Bass program. nblk = edge blocks per tile."""
    nc = bacc.Bacc(
        "TRN2",
        target_bir_lowering=False,
        debug=False,
        enable_asserts=False,
        num_devices=NCORES,
    )
    L = nblk * 128  # padded edges per destination tile

    # ---- inputs ----
    x_in = nc.dram_tensor("x", [N, IN], FP, kind="ExternalInput")
    xT_own = nc.dram_tensor("xT_own", [128, OWN], FP, kind="ExternalInput")
    degs_in = nc.dram_tensor("degs", [128, TPC * nblk], FP, kind="ExternalInput")
    deg_own_in = nc.dram_tensor("deg_own", [128, TPC], FP, kind="ExternalInput")
    w1_in = nc.dram_tensor("W1", [IN, H1], FP, kind="ExternalInput")
    w2_in = nc.dram_tensor("W2", [H1, H2], FP, kind="ExternalInput")
    w3_in = nc.dram_tensor("W3p", [H2, LATP], FP, kind="ExternalInput")
    skw_in = nc.dram_tensor("skWp", [IN, LATP], FP, kind="ExternalInput")
    sc1_in = nc.dram_tensor("sc1", [128, H1], FP, kind="ExternalInput")
    sh1_in = nc.dram_tensor("sh1", [128, H1], FP, kind="ExternalInput")
    b2b_in = nc.dram_tensor("b2b", [128, H2], FP, kind="ExternalInput")
    zbb_in = nc.dram_tensor("zbb", [128, LATP], FP, kind="ExternalInput")
    dw1_in = nc.dram_tensor("dW1p", [LATP, LATP], FP, kind="ExternalInput")
    db1_in = nc.dram_tensor("db1b", [128, LATP], FP, kind="ExternalInput")
    dw2_in = nc.dram_tensor("dw2b", [128, LATP], FP, kind="ExternalInput")
    src_in = nc.dram_tensor("src16", [128, TPC * nblk * 8], I16, kind="ExternalInput")
    dst_in = nc.dram_tensor("dstf", [128, TPC * nblk], FP, kind="ExternalInput")

    # ---- outputs ----
    z_out = nc.dram_tensor("z_out", [OWN, LAT], FP, kind="ExternalOutput")
    adj_out = nc.dram_tensor("adj_out", [OWN, N], FP, kind="ExternalOutput")
    dp_out = nc.dram_tensor("dp_out", [OWN, 1], FP, kind="ExternalOutput")

    groups = [list(range(NCORES))]

    with tile.TileContext(nc) as tc:
        with (
            tc.tile_pool(name="const", bufs=1) as const,
            tc.tile_pool(name="dram", bufs=1, space="DRAM") as dram,
        ):
            # persistent DRAM scratch
            m2_bounce = dram.tile([OWN, H2], FP, tag="m2b")
            m2_tab = dram.tile([N, H2], FP, tag="m2t")
            m3_bounce = dram.tile([OWN, LATP], FP, tag="m3b")
            m3_tab = dram.tile([N, LATP], FP, tag="m3t")
            BF = mybir.dt.bfloat16
            zT_bounce_hi = dram.tile([LATP, OWN], BF, tag="ztbh")
            zT_bounce_lo = dram.tile([LATP, OWN], BF, tag="ztbl")
            zT_cat_hi = dram.tile([NCORES * LATP, OWN], BF, tag="ztch")
            zT_cat_lo = dram.tile([NCORES * LATP, OWN], BF, tag="ztcl")

            # ---- constants in SBUF ----
            iota_f = const.tile([128, 128], FP, tag="iota")
            nc.gpsimd.iota(
                iota_f[:], pattern=[[1, 128]], base=0, channel_multiplier=0,
                allow_small_or_imprecise_dtypes=True,
            )
            iota_pm = const.tile([128, 128], I32, tag="iotapm")
            nc.gpsimd.iota(
                iota_pm[:], pattern=[[-1, 128]], base=0, channel_multiplier=1
            )
            ident = const.tile([128, 128], FP, tag="ident")
            nc.vector.tensor_scalar(ident[:], iota_pm[:], 0, None, op0=ALU.is_equal)

            w1_sb = const.tile([128, H1], FP, tag="w1")
            nc.sync.dma_start(w1_sb[:], w1_in[:])
            w2_sb = const.tile([128, 2, H2], FP, tag="w2")
            nc.sync.dma_start(w2_sb[:], w2_in.rearrange("(k p) n -> p k n", p=128))
            w3_sb = const.tile([128, LATP], FP, tag="w3")
            nc.sync.dma_start(w3_sb[:], w3_in[:])
            skw_sb = const.tile([128, LATP], FP, tag="skw")
            nc.sync.dma_start(skw_sb[:], skw_in[:])
            sc1_sb = const.tile([128, H1], FP, tag="sc1")
            nc.sync.dma_start(sc1_sb[:], sc1_in[:])
            sh1_sb = const.tile([128, H1], FP, tag="sh1")
            nc.sync.dma_start(sh1_sb[:], sh1_in[:])
            b2b_sb = const.tile([128, H2], FP, tag="b2b")
            nc.sync.dma_start(b2b_sb[:], b2b_in[:])
            zbb_sb = const.tile([128, LATP], FP, tag="zbb")
            nc.sync.dma_start(zbb_sb[:], zbb_in[:])
            dw1_sb = const.tile([LATP, LATP], FP, tag="dw1")
            nc.sync.dma_start(dw1_sb[:], dw1_in[:])
            db1_sb = const.tile([128, LATP], FP, tag="db1")
            nc.sync.dma_start(db1_sb[:], db1_in[:])
            dw2_sb = const.tile([128, LATP], FP, tag="dw2")
            nc.sync.dma_start(dw2_sb[:], dw2_in[:])
            xTo_sb = const.tile([128, OWN], FP, tag="xto")
            nc.sync.dma_start(xTo_sb[:], xT_own[:])
            src_sb = const.tile([128, TPC * nblk * 8], I16, tag="src")
            nc.sync.dma_start(src_sb[:], src_in[:])
            dst_sb = const.tile([128, TPC * nblk], FP, tag="dst")
            nc.sync.dma_start(dst_sb[:], dst_in[:])

            degs_sb = const.tile([128, TPC * nblk], FP, tag="degs")
            nc.sync.dma_start(degs_sb[:], degs_in[:])
            dsrc_inv = const.tile([128, TPC * nblk], FP, tag="dsrcinv")
            nc.vector.reciprocal(dsrc_inv[:], degs_sb[:])
            nc.scalar.activation(dsrc_inv[:], dsrc_inv[:], AF.Sqrt)

            dego_sb = const.tile([128, TPC], FP, tag="dego")
            nc.sync.dma_start(dego_sb[:], deg_own_in[:])
            dinv_own = const.tile([128, TPC], FP, tag="dinvown")
            nc.vector.reciprocal(dinv_own[:], dego_sb[:])
            nc.scalar.activation(dinv_own[:], dinv_own[:], AF.Sqrt)

            zT_own = const.tile([LATP, TPC, 128], FP, tag="ztown")

            # ---- generic aggregation: psum_agg += S_b^T @ gathered_b ----
            iota_b = iota_f[:].rearrange("p (a f) -> p a f", a=1).broadcast_to(
                (128, nblk, 128)
            )

            def aggregate(sb, ps, tab, width, t, sfold=None):
                g = sb.tile([128, nblk, width], FP, tag="gather")
                nc.gpsimd.dma_gather(
                    g[:],
                    tab[:],
                    src_sb[:, t * nblk * 8:(t + 1) * nblk * 8],
                    L,
                    L,
                    width,
                    single_packet=False,
                )
                # all NBLK selector matrices in one DVE op:
                # S[p, b, j] = (j == dst_local[p, b])
                S = sb.tile([128, nblk, 128], FP, tag="S")
                nc.vector.tensor_tensor(
                    S[:],
                    iota_b,
                    dst_sb[:, t * nblk:(t + 1) * nblk].broadcast_to(
                        (128, nblk, 128)
                    ),
                    op=ALU.is_equal,
                )
                if sfold is not None:
                    nc.vector.tensor_tensor(
                        S[:], S[:],
                        sfold[:, t * nblk:(t + 1) * nblk].broadcast_to(
                            (128, nblk, 128)
                        ),
                        op=ALU.mult,
                    )
                agg = ps.tile([128, width], FP, tag="agg")
                for b in range(nblk):
                    nc.tensor.matmul(
                        agg[:], S[:, b, :], g[:, b, :],
                        start=(b == 0), stop=(b == nblk - 1),
                    )
                return agg

            # ---- conv1: agg x~ -> @W1 -> BN+ReLU -> @W2 -> m2~ ----
            with (
                tc.tile_pool(name="c1sb", bufs=3) as sb,
                tc.tile_pool(name="c1ps", bufs=2, space="PSUM") as ps,
            ):
                for t in range(TPC):
                    agg = aggregate(sb, ps, x_in, IN, t, sfold=dsrc_inv)
                    aggs = sb.tile([128, IN], FP, tag="aggs")
                    nc.vector.tensor_scalar(
                        aggs[:], agg[:], dinv_own[:, t:t + 1], None, op0=ALU.mult
                    )
                    aT_ps = ps.tile([128, 128], FP, tag="tp")
                    nc.tensor.transpose(aT_ps[:], aggs[:], ident[:])
                    aT = sb.tile([128, 128], FP, tag="aT")
                    nc.scalar.activation(aT[:], aT_ps[:], AF.Copy)
                    h1ps = ps.tile([128, H1], FP, tag="h1ps")
                    nc.tensor.matmul(h1ps[:], aT[:], w1_sb[:], start=True, stop=True)
                    t1 = sb.tile([128, H1], FP, tag="t1")
                    nc.vector.tensor_mul(t1[:], h1ps[:], sc1_sb[:])
                    t2 = sb.tile([128, H1], FP, tag="t2")
                    nc.vector.tensor_add(t2[:], t1[:], sh1_sb[:])
                    h1 = sb.tile([128, H1], FP, tag="h1")
                    nc.scalar.activation(h1[:], t2[:], AF.Relu)
                    m2ps = ps.tile([128, H2], FP, tag="m2ps")
                    for k in range(2):
                        hT_ps = ps.tile([128, 128], FP, tag="tp")
                        nc.tensor.transpose(
                            hT_ps[:], h1[:, k * 128:(k + 1) * 128], ident[:]
                        )
                        hT = sb.tile([128, 128], FP, tag="hT")
                        nc.scalar.activation(hT[:], hT_ps[:], AF.Copy)
                        nc.tensor.matmul(
                            m2ps[:], hT[:], w2_sb[:, k, :],
                            start=(k == 0), stop=(k == 1),
                        )
                    m2s = sb.tile([128, H2], FP, tag="m2s")
                    nc.vector.tensor_scalar(
                        m2s[:], m2ps[:], dinv_own[:, t:t + 1], None, op0=ALU.mult
                    )
                    nc.sync.dma_start(m2_bounce[t * 128:(t + 1) * 128, :], m2s[:])

            if phases >= 2:
                nc.gpsimd.collective_compute(
                    "AllGather", ALU.bypass, replica_groups=groups,
                    ins=[m2_bounce.opt()], outs=[m2_tab.opt()],
                )

                # ---- conv2: agg m2~ -> +b2,ReLU -> @W3 -> m3~ ----
                with (
                    tc.tile_pool(name="c2sb", bufs=3) as sb,
                    tc.tile_pool(name="c2ps", bufs=2, space="PSUM") as ps,
                ):
                    for t in range(TPC):
                        agg = aggregate(sb, ps, m2_tab, H2, t)
                        a1 = sb.tile([128, H2], FP, tag="a1")
                        nc.vector.tensor_scalar(
                            a1[:], agg[:], dinv_own[:, t:t + 1], None, op0=ALU.mult
                        )
                        a2 = sb.tile([128, H2], FP, tag="a2")
                        nc.vector.tensor_add(a2[:], a1[:], b2b_sb[:])
                        h2 = sb.tile([128, H2], FP, tag="h2")
                        nc.scalar.activation(h2[:], a2[:], AF.Relu)
                        hT_ps = ps.tile([128, 128], FP, tag="tp")
                        nc.tensor.transpose(hT_ps[:], h2[:], ident[:])
                        hT = sb.tile([128, 128], FP, tag="hT")
                        nc.scalar.activation(hT[:], hT_ps[:], AF.Copy)
                        m3ps = ps.tile([128, LATP], FP, tag="m3ps")
                        nc.tensor.matmul(
                            m3ps[:], hT[:], w3_sb[:], start=True, stop=True
                        )
                        m3s = sb.tile([128, LATP], FP, tag="m3s")
                        nc.vector.tensor_scalar(
                            m3s[:], m3ps[:], dinv_own[:, t:t + 1], None, op0=ALU.mult
                        )
                        nc.sync.dma_start(
                            m3_bounce[t * 128:(t + 1) * 128, :], m3s[:]
                        )

            if phases >= 3:
                nc.gpsimd.collective_compute(
                    "AllGather", ALU.bypass, replica_groups=groups,
                    ins=[m3_bounce.opt()], outs=[m3_tab.opt()],
                )

                # ---- conv3: agg m3~ -> +zb + identity -> z; degree head ----
                with (
                    tc.tile_pool(name="c3sb", bufs=3) as sb,
                    tc.tile_pool(name="c3ps", bufs=2, space="PSUM") as ps,
                ):
                    for t in range(TPC):
                        agg = aggregate(sb, ps, m3_tab, LATP, t)
                        za = sb.tile([128, LATP], FP, tag="za")
                        nc.vector.tensor_scalar(
                            za[:], agg[:], dinv_own[:, t:t + 1], None, op0=ALU.mult
                        )
                        idps = ps.tile([128, LATP], FP, tag="idps")
                        nc.tensor.matmul(
                            idps[:], xTo_sb[:, t * 128:(t + 1) * 128], skw_sb[:],
                            start=True, stop=True,
                        )
                        zb = sb.tile([128, LATP], FP, tag="zb")
                        nc.vector.tensor_add(zb[:], za[:], idps[:])
                        zt = sb.tile([128, LATP], FP, tag="zt")
                        nc.vector.tensor_add(zt[:], zb[:], zbb_sb[:])
                        nc.sync.dma_start(
                            z_out[t * 128:(t + 1) * 128, :], zt[:, :LAT]
                        )
                        zT_ps = ps.tile([LATP, 128], FP, tag="ztps")
                        nc.tensor.transpose(zT_ps[:], zt[:], ident[:])
                        nc.scalar.activation(zT_own[:, t, :], zT_ps[:], AF.Copy)
                        # split z^T into bf16 hi + lo for the adj matmuls
                        hi_bf = sb.tile([LATP, 128], BF, tag="hibf")
                        nc.scalar.activation(hi_bf[:], zT_ps[:], AF.Copy)
                        hi32 = sb.tile([LATP, 128], FP, tag="hi32")
                        nc.scalar.activation(hi32[:], hi_bf[:], AF.Copy)
                        lo32 = sb.tile([LATP, 128], FP, tag="lo32")
                        nc.vector.tensor_sub(lo32[:], zT_ps[:], hi32[:])
                        lo_bf = sb.tile([LATP, 128], BF, tag="lobf")
                        nc.scalar.activation(lo_bf[:], lo32[:], AF.Copy)
                        nc.sync.dma_start(
                            zT_bounce_hi[:, t * 128:(t + 1) * 128], hi_bf[:]
                        )
                        nc.sync.dma_start(
                            zT_bounce_lo[:, t * 128:(t + 1) * 128], lo_bf[:]
                        )
                        # degree head: relu(z @ dpW1 + dpb1) @ dpW2 + dpb2
                        hd_ps = ps.tile([128, LATP], FP, tag="hdps")
                        nc.tensor.matmul(
                            hd_ps[:], zT_own[:, t, :], dw1_sb[:],
                            start=True, stop=True,
                        )
                        hd1 = sb.tile([128, LATP], FP, tag="hd1")
                        nc.vector.tensor_add(hd1[:], hd_ps[:], db1_sb[:])
                        hd = sb.tile([128, LATP], FP, tag="hd")
                        nc.scalar.activation(hd[:], hd1[:], AF.Relu)
                        hm = sb.tile([128, LATP], FP, tag="hm")
                        nc.vector.tensor_mul(hm[:], hd[:], dw2_sb[:])
                        dp = sb.tile([128, 1], FP, tag="dp")
                        nc.vector.tensor_reduce(
                            dp[:], hm[:], axis=mybir.AxisListType.X, op=ALU.add
                        )
                        dp2 = sb.tile([128, 1], FP, tag="dp2")
                        nc.vector.tensor_scalar(
                            dp2[:], dp[:], float(dpb2_val), None, op0=ALU.add
                        )
                        nc.sync.dma_start(
                            dp_out[t * 128:(t + 1) * 128, :], dp2[:]
                        )

            if phases >= 4:
                nc.gpsimd.collective_compute(
                    "AllGather", ALU.bypass, replica_groups=groups,
                    ins=[zT_bounce_hi.opt()], outs=[zT_cat_hi.opt()],
                )
                nc.gpsimd.collective_compute(
                    "AllGather", ALU.bypass, replica_groups=groups,
                    ins=[zT_bounce_lo.opt()], outs=[zT_cat_lo.opt()],
                )

                # ---- adj_recon = sigmoid(z @ z.T), row block per core ----
                # split-bf16: logits = hi@hi + lo@hi + hi@lo  (lo@lo dropped)
                # MM_A: lhsT=[hi;lo], rhs=[hi;hi];  MM_B: lhsT=[hi;0], rhs=[lo;lo]
                with (
                    tc.tile_pool(name="a4sb", bufs=3) as sb,
                    tc.tile_pool(name="a4c", bufs=1) as a4c,
                    tc.tile_pool(name="a4ps", bufs=8, space="PSUM") as ps,
                ):
                    zall_hh = a4c.tile([128, NCORES, OWN], BF, tag="zallhh")
                    zall_ll = a4c.tile([128, NCORES, OWN], BF, tag="zallll")
                    for cb in range(NCORES):
                        cat_h = zT_cat_hi[cb * LATP:(cb + 1) * LATP, :]
                        cat_l = zT_cat_lo[cb * LATP:(cb + 1) * LATP, :]
                        nc.sync.dma_start(zall_hh[0:LATP, cb, :], cat_h)
                        nc.sync.dma_start(zall_hh[LATP:128, cb, :], cat_h)
                        nc.sync.dma_start(zall_ll[0:LATP, cb, :], cat_l)
                        nc.sync.dma_start(zall_ll[LATP:128, cb, :], cat_l)
                    zown_A = a4c.tile([128, TPC, 128], BF, tag="zownA")
                    zown_B = a4c.tile([128, TPC, 128], BF, tag="zownB")
                    nc.sync.dma_start(
                        zown_A[0:LATP, :, :],
                        zT_bounce_hi[:].rearrange("p (t f) -> p t f", f=128),
                    )
                    nc.sync.dma_start(
                        zown_A[LATP:128, :, :],
                        zT_bounce_lo[:].rearrange("p (t f) -> p t f", f=128),
                    )
                    nc.sync.dma_start(
                        zown_B[0:LATP, :, :],
                        zT_bounce_hi[:].rearrange("p (t f) -> p t f", f=128),
                    )
                    nc.gpsimd.memset(zown_B[LATP:128, :, :], 0.0)

                    NJ = OWN // 512
                    for r in range(TPC):
                        for cb in range(NCORES):
                            orow = sb.tile([128, OWN], FP, tag="orow")
                            apss = []
                            for _j in range(NJ):
                                aps_t = ps.tile([128, 512], FP, tag="adjps")
                                apss.append(aps_t)
                            for j in range(NJ):
                                nc.tensor.matmul(
                                    apss[j][:],
                                    zown_A[:, r, :],
                                    zall_hh[:, cb, j * 512:(j + 1) * 512],
                                    start=True, stop=False,
                                )
                            for j in range(NJ):
                                nc.tensor.matmul(
                                    apss[j][:],
                                    zown_B[:, r, :],
                                    zall_ll[:, cb, j * 512:(j + 1) * 512],
                                    start=False, stop=True,
                                )
                            for j in range(NJ):
                                nc.scalar.activation(
                                    orow[:, j * 512:(j + 1) * 512], apss[j][:],
                                    AF.Sigmoid,
                                )
                            nc.sync.dma_start(
                                adj_out[
                                    r * 128:(r + 1) * 128, cb * OWN:(cb + 1) * OWN
                                ],
                                orow[:],
                            )

    nc.compile()
    return nc


def _prep_host(x, edge_index, W1, b1, W2, b2, W3, b3, skip_W, skip_b,
               bn_gamma, bn_beta, bn_mean, bn_var, dpW1, dpb1, dpW2, dpb2):
    x = np.asarray(x, np.float32)
    ei = np.asarray(edge_index)
    loops = np.arange(N, dtype=ei.dtype)
    src = np.concatenate([ei[0], loops]).astype(np.int64)
    dst = np.concatenate([ei[1], loops]).astype(np.int64)
    deg = np.bincount(dst, minlength=N).astype(np.float32)

    order = np.argsort(dst, kind="stable")
    ssrc = src[order]
    sdst = dst[order]
    tile_of = sdst >> 7
    counts = np.bincount(tile_of, minlength=NT)
    nblk = int(np.ceil(counts.max() / 128))
    L = nblk * 128

    # per destination tile: padded src (int16) and local dst (f32, 999 pad)
    src_pad = np.zeros((NT, L), np.int16)
    dst_pad = np.full((NT, L), 999.0, np.float32)
    starts = np.zeros(NT + 1, np.int64)
    np.cumsum(counts, out=starts[1:])
    for t in range(NT):
        s, e = starts[t], starts[t + 1]
        cnt = e - s
        src_pad[t, :cnt] = ssrc[s:e].astype(np.int16)
        dst_pad[t, :cnt] = (sdst[s:e] - t * 128).astype(np.float32)

    W1 = np.asarray(W1, np.float32)
    W2 = np.asarray(W2, np.float32)
    W3 = np.asarray(W3, np.float32)
    skip_W = np.asarray(skip_W, np.float32)
    sc1 = (np.asarray(bn_gamma) / np.sqrt(np.asarray(bn_var) + EPS)).astype(np.float32)
    sh1 = ((np.asarray(b1) - np.asarray(bn_mean)) * sc1 + np.asarray(bn_beta)).astype(np.float32)
    W3p = np.zeros((H2, LATP), np.float32)
    W3p[:, :LAT] = W3
    skWp = np.zeros((IN, LATP), np.float32)
    skWp[:, :LAT] = skip_W
    zb = np.zeros(LATP, np.float32)
    zb[:LAT] = np.asarray(b3, np.float32) + np.asarray(skip_b, np.float32)
    dW1p = np.zeros((LATP, LATP), np.float32)
    dW1p[:LAT, :] = np.asarray(dpW1, np.float32)
    db1 = np.asarray(dpb1, np.float32)
    dw2 = np.asarray(dpW2, np.float32)[:, 0]
    dpb2_val = float(np.asarray(dpb2)[0])

    def bcast(v, w):
        return np.broadcast_to(np.asarray(v, np.float32)[None, :], (128, w)).copy()

    deg_pt = deg.reshape(NT, 128).T.copy()
    degs_pad = deg[src_pad.astype(np.int64)]          # [NT, L] = deg[src_e]

    common = {
        "x": x,
        "W1": W1,
        "W2": W2,
        "W3p": W3p,
        "skWp": skWp,
        "sc1": bcast(sc1, H1),
        "sh1": bcast(sh1, H1),
        "b2b": bcast(b2, H2),
        "zbb": bcast(zb, LATP),
        "dW1p": dW1p,
        "db1b": bcast(db1, LATP),
        "dw2b": bcast(dw2, LATP),
    }

    in_maps = []
    for c in range(NCORES):
        t0 = c * TPC
        sp = src_pad[t0:t0 + TPC].reshape(-1)          # [TPC*L]
        dp = dst_pad[t0:t0 + TPC]                      # [TPC, L]
        m = dict(common)
        m["xT_own"] = x[c * OWN:(c + 1) * OWN].T.copy()
        m["deg_own"] = deg_pt[:, t0:t0 + TPC].copy()
        # wrapped in 16 partitions, replicated for each of the 8 Q7 cores
        m["src16"] = np.tile(sp.reshape(-1, 16).T, (8, 1)).copy()  # [128, TPC*L/16]
        m["dstf"] = dp.reshape(TPC * nblk, 128).T.copy()  # [128, TPC*nblk]
        dg = degs_pad[t0:t0 + TPC]
        m["degs"] = dg.reshape(TPC * nblk, 128).T.copy()
        in_maps.append(m)
    return nblk, dpb2_val, in_maps


def _run(nc, in_maps):
    return bass_utils.run_bass_kernel_spmd(
        nc, in_maps, core_ids=list(range(NCORES))
    )


def kernel(**inputs):
    import os

    phases = int(os.environ.get("GAE_PHASES", "4"))
    nblk, dpb2_val, in_maps = _prep_host(**inputs)
    key = (nblk, dpb2_val, phases)
    if key not in _COMPILED:
        _COMPILED[key] = _build(nblk, dpb2_val, phases)
    nc = _COMPILED[key]
    res = _run(nc, in_maps)
    z = np.concatenate([res.results[c]["z_out"] for c in range(NCORES)], axis=0)
    adj = np.concatenate([res.results[c]["adj_out"] for c in range(NCORES)], axis=0)
    dp = np.concatenate(
        [res.results[c]["dp_out"][:, 0] for c in range(NCORES)], axis=0
    )
    return z, adj, dp
